# revision 3
# baseline (speedup 1.0000x reference)
"""Differential attention Trainium2 kernel (Bass/Tile), 8-core SPMD.

reference:
  attn1 = softmax(causal(Q1 K1^T / sqrt(D))) V
  attn2 = softmax(causal(Q2 K2^T / sqrt(D))) V
  out   = attn1 - exp(lambda_log) * attn2
shapes: [B=2, H=12, S=2048, D=128] fp32.

Sharding: B*H = 24 head-batches, 3 per NeuronCore (data/head parallel, no
cross-core comms). Host pre-transposes Q/K to [D, S] layout so the device
needs no on-chip transposes; device returns output d-major ([D, S] per
head) and the host transposes back.

Matmul dtype strategy: all matmul operands (Q^T, K^T, V, exp-scores E,
ones) are fp16 (PE streams 1 col/cycle for 2-byte dtypes; fp32 is 4x
slower; fp8 would halve precision below the tolerance). PSUM accumulation
stays fp32 and lambda is applied exactly in fp32. Error ~4e-4 of absmax.

Device algorithm per (head, pass), in score-transposed layout:
  S_T[k, q] = matmul(lhsT=K^T_j, rhs=Q^T[q-group])      (contract D)
  E_T = exp(SCALE * S_T)  fp16   (ScalarE, PSUM->SBUF)
  out_T[d, q] += matmul(lhsT=V_j, rhs=E_T)              (contract k, PSUM acc)
  sums[128, q] += matmul(lhsT=ones128, rhs=E_T)         (denominator,
                                  pre-broadcast across all partitions)
then fin = out1_T*recip(sums1) - lam*(out2_T*recip(sums2)), muls/recip on
DVE, the final combine on the otherwise-idle GpSimd engine.

Perf structure (v2):
  - score tiles are [128,1024] fp32 = 2 PSUM banks; full key-tiles are
    processed in j-PAIRS with a single exp per pair (the ACT engine has a
    ~260ns fixed cost per instruction; halving the instruction count saves
    ~20us of ScalarE time).
  - the 4 diagonal tiles are packed into two tiles: (dr0|dr1) -> 512+384
    cols, (dr2|dr3) -> 256+128 cols, each with ONE exp and ONE two-band
    copy_predicated (bands have uniform stride inside each packed tile).
  - per-head DMA loads are split critical-first: the [0:512] slices of
    Q/K (which gate group g=0) and the first 4 V tiles land before the
    tails; the tiny const loads (ones/tri/neglam) are issued after head
    0's critical slices so they don't delay the first matmul.
"""

import sys

sys.path.insert(0, "/opt/trn_rl_repo")

import numpy as np

B, H, S, D = 2, 12, 2048, 128
NCORES = 8
BH = B * H
HEADS = BH // NCORES  # 3 heads per core
P = 128
NT = S // P           # 16 key tiles
GW = 512              # query-group width (matmul free dim)
G = S // GW           # 4 query groups
TPG = GW // P         # 4 tiles per group
SCALE = float(D) ** -0.5

_PROGRAM = None


def _build_program():
    import concourse.mybir as mybir
    import concourse.tile as tile
    from concourse import bacc

    fp32 = mybir.dt.float32
    fp16 = mybir.dt.float16
    u8 = mybir.dt.uint8
    Exp = mybir.ActivationFunctionType.Exp
    Mult = mybir.AluOpType.mult
    Add = mybir.AluOpType.add

    nc = bacc.Bacc(None)
    qt1 = nc.dram_tensor("qt1", [HEADS, P, S], fp16, kind="ExternalInput")
    kt1 = nc.dram_tensor("kt1", [HEADS, P, S], fp16, kind="ExternalInput")
    qt2 = nc.dram_tensor("qt2", [HEADS, P, S], fp16, kind="ExternalInput")
    kt2 = nc.dram_tensor("kt2", [HEADS, P, S], fp16, kind="ExternalInput")
    vd = nc.dram_tensor("v", [HEADS, P, NT, D], fp16, kind="ExternalInput")
    neglam = nc.dram_tensor("neglam", [P, 1], fp32, kind="ExternalInput")
    onesd = nc.dram_tensor("ones", [P, P], fp16, kind="ExternalInput")
    tri = nc.dram_tensor("tri", [P, P], u8, kind="ExternalInput")
    out = nc.dram_tensor("out", [HEADS, P, S], fp32, kind="ExternalOutput")

    with tile.TileContext(nc) as tc:
        with (
            tc.tile_pool(name="const", bufs=1) as cpool,
            tc.tile_pool(name="load", bufs=3) as lpool,
            tc.tile_pool(name="et", bufs=3) as epool,
            tc.tile_pool(name="fin", bufs=4) as fpool,
            tc.tile_pool(name="spsum", bufs=1, space="PSUM") as spool,
            tc.tile_pool(name="opsum", bufs=1, space="PSUM") as opool,
            tc.tile_pool(name="supsum", bufs=1, space="PSUM") as upool,
        ):
            tri_s = cpool.tile([P, P], u8)
            negbig = cpool.tile([P, P], fp32)
            nc.vector.memset(negbig[:], -1.0e30)
            neglam_s = cpool.tile([P, 1], fp32)
            ones_mat = cpool.tile([P, P], fp16)

            for h in range(HEADS):
                # critical-first loads: group g=0 needs q[0:GW], k[0:GW]
                # (diag j=0..3) and the first TPG V tiles
                qk = []
                for name, t in (("q1", qt1), ("k1", kt1), ("q2", qt2), ("k2", kt2)):
                    ts_ = lpool.tile([P, S], fp16, tag=name)
                    nc.sync.dma_start(ts_[:, 0:GW], t[h][:, 0:GW])
                    qk.append(ts_)
                v_s = lpool.tile([P, NT, D], fp16, tag="v")
                nc.sync.dma_start(v_s[:, 0:TPG, :], vd[h][:, 0:TPG, :])
                if h == 0:
                    nc.sync.dma_start(ones_mat[:], onesd[:])
                    nc.sync.dma_start(tri_s[:], tri[:])
                    nc.sync.dma_start(neglam_s[:], neglam[:])
                # tails: k first (g=1 needs k up to 1024), then v, then q
                nc.sync.dma_start(qk[1][:, GW:], kt1[h][:, GW:])
                nc.sync.dma_start(qk[3][:, GW:], kt2[h][:, GW:])
                nc.sync.dma_start(v_s[:, TPG:, :], vd[h][:, TPG:, :])
                nc.sync.dma_start(qk[0][:, GW:], qt1[h][:, GW:])
                nc.sync.dma_start(qk[2][:, GW:], qt2[h][:, GW:])

                for g in range(G):
                    jfull = TPG * g
                    qcols = [qk[2 * pi][:, g * GW : (g + 1) * GW] for pi in range(2)]
                    kss = [qk[2 * pi + 1] for pi in range(2)]
                    outp = [
                        opool.tile([P, GW], fp32, tag=f"outp{pi}", name=f"outp{pi}_{h}_{g}")
                        for pi in range(2)
                    ]
                    sums = [
                        upool.tile([P, GW], fp32, tag=f"sums{pi}", name=f"sums{pi}_{h}_{g}")
                        for pi in range(2)
                    ]
                    # full key-tiles in pairs: one [128,1024] score tile
                    # (2 PSUM banks) and ONE exp per (pair, pass)
                    for jp in range(jfull // 2):
                        j0, j1 = 2 * jp, 2 * jp + 1
                        sts, ets = [], []
                        for pi in range(2):
                            st = spool.tile([P, 2 * GW], fp32, tag=f"st{pi}")
                            et = epool.tile([P, 2 * GW], fp16, tag=f"et{pi}")
                            for jj, j in enumerate((j0, j1)):
                                nc.tensor.matmul(
                                    st[:, jj * GW : (jj + 1) * GW],
                                    kss[pi][:, j * P : (j + 1) * P],
                                    qcols[pi],
                                    start=True,
                                    stop=True,
                                )
                            sts.append(st)
                            ets.append(et)
                        for pi in range(2):
                            nc.scalar.activation(
                                ets[pi][:], sts[pi][:], Exp, scale=SCALE
                            )
                        for pi in range(2):
                            for jj, j in enumerate((j0, j1)):
                                ecols = ets[pi][:, jj * GW : (jj + 1) * GW]
                                nc.tensor.matmul(
                                    sums[pi][:], ones_mat[:], ecols,
                                    start=(j == 0), stop=False,
                                )
                                nc.tensor.matmul(
                                    outp[pi][:], v_s[:, j, :], ecols,
                                    start=(j == 0), stop=False,
                                )
                    # diagonal tiles dr=0..3 (j = jfull+dr), shrunk to the
                    # surviving n = 512-128*dr cols, packed in two tiles:
                    # (dr0|dr1) at offsets 0,512 and (dr2|dr3) at 0,256.
                    # Band stride inside each tile is uniform -> single
                    # two-band copy_predicated per tile.
                    for grp in ((0, 1), (2, 3)):
                        for pi in range(2):
                            ks = kss[pi]
                            st = spool.tile([P, 2 * GW], fp32, tag=f"st{pi}")
                            et = epool.tile([P, 2 * GW], fp16, tag=f"et{pi}")
                            blk = GW - grp[0] * P  # width of first region
                            regions = []
                            off = 0
                            for dr in grp:
                                j = jfull + dr
                                col0 = dr * P      # q offset in group
                                n = GW - col0
                                regions.append((j, dr, col0, n, off))
                                nc.tensor.matmul(
                                    st[:, off : off + n],
                                    ks[:, j * P : (j + 1) * P],
                                    qk[2 * pi][:, g * GW + col0 : (g + 1) * GW],
                                    start=True,
                                    stop=True,
                                )
                                off += blk
                            # causal band: first 128 cols of each region
                            # (regions start at 0 and blk -> stride blk)
                            bands = st[:, 0 : 2 * blk].rearrange(
                                "p (b c) -> p b c", b=2, c=blk
                            )[:, :, 0:P]
                            nc.vector.copy_predicated(
                                bands,
                                tri_s[:]
                                .rearrange("p c -> p () c")
                                .broadcast_to([P, 2, P]),
                                negbig[:]
                                .rearrange("p c -> p () c")
                                .broadcast_to([P, 2, P]),
                            )
                            width = blk + (GW - grp[1] * P)
                            nc.scalar.activation(
                                et[:, :width], st[:, :width], Exp, scale=SCALE
                            )
                            for j, dr, col0, n, roff in regions:
                                ecols = et[:, roff : roff + n]
                                nc.tensor.matmul(
                                    sums[pi][:, col0:], ones_mat[:], ecols,
                                    start=(dr == 0 and jfull == 0),
                                    stop=(dr == TPG - 1),
                                )
                                nc.tensor.matmul(
                                    outp[pi][:, col0:], v_s[:, j, :], ecols,
                                    start=(dr == 0 and jfull == 0),
                                    stop=(dr == TPG - 1),
                                )
                    # epilogue: rcp0 -> mul0 (frees sums0/outp0 for the next
                    # group ASAP) -> rcp1 -> mul1; combine on GpSimd
                    rcps, ts = [], []
                    for pi in range(2):
                        rcp = fpool.tile([P, GW], fp32, tag=f"rcp{pi}")
                        scr = fpool.tile([P, GW], fp32, tag="scr")
                        nc.vector.reciprocal_approx_accurate(
                            rcp[:], sums[pi][:], scr[:]
                        )
                        t_ = fpool.tile([P, GW], fp32, tag=f"t{pi}")
                        nc.vector.tensor_mul(t_[:], outp[pi][:], rcp[:])
                        rcps.append(rcp)
                        ts.append(t_)
                    fin = fpool.tile([P, GW], fp32, tag="fin")
                    # fin = t0 - lam*t1  (lam exact in fp32 via neglam column)
                    nc.vector.scalar_tensor_tensor(
                        fin[:], ts[1][:], neglam_s[:], ts[0][:],
                        op0=Mult, op1=Add,
                    )
                    nc.sync.dma_start(out[h][:, g * GW : (g + 1) * GW], fin[:])

    nc.compile()
    return nc


def _get_program():
    global _PROGRAM
    if _PROGRAM is None:
        _PROGRAM = _build_program()
    return _PROGRAM


def _make_in_maps(q1, k1, v, q2, k2, lambda_log):
    lam_val = float(np.exp(np.float64(lambda_log.reshape(-1)[0])))
    neglam_np = np.full((P, 1), -lam_val, dtype=np.float32)
    ones_np = np.ones((P, P), dtype=np.float16)
    # kill-mask for the diagonal band: 1 where k > q (strictly below diag)
    tri_np = (np.arange(P)[:, None] > np.arange(P)[None, :]).astype(np.uint8)

    def t(x, dt_):  # [BH, S, D] -> [BH, D, S] contiguous
        return np.ascontiguousarray(
            x.reshape(BH, S, D).transpose(0, 2, 1)
        ).astype(dt_)

    q1t = t(q1, np.float16)
    q2t = t(q2, np.float16)
    k1t = t(k1, np.float16)
    k2t = t(k2, np.float16)
    # pre-tile V to [BH, p, j, d] so the SBUF load is contiguous per
    # partition: v_s[p, j, d] = V[128 j + p, d]
    vf = np.ascontiguousarray(
        v.reshape(BH, NT, P, D).transpose(0, 2, 1, 3)
    ).astype(np.float16)

    in_maps = []
    for c in range(NCORES):
        sl = slice(c * HEADS, (c + 1) * HEADS)
        in_maps.append(
            {
                "qt1": q1t[sl],
                "kt1": k1t[sl],
                "qt2": q2t[sl],
                "kt2": k2t[sl],
                "v": vf[sl],
                "neglam": neglam_np,
                "ones": ones_np,
                "tri": tri_np,
            }
        )
    return in_maps


def _run(q1, k1, v, q2, k2, lambda_log, trace=False):
    from concourse.bass_utils import run_bass_kernel_spmd

    nc = _get_program()
    in_maps = _make_in_maps(q1, k1, v, q2, k2, lambda_log)
    res = run_bass_kernel_spmd(
        nc, in_maps, core_ids=list(range(NCORES)), trace=trace
    )
    parts = [res.results[c]["out"].transpose(0, 2, 1) for c in range(NCORES)]
    full = np.concatenate(parts, axis=0).reshape(B, H, S, D)
    return np.ascontiguousarray(full, dtype=np.float32), res


def kernel(q1, k1, v, q2, k2, lambda_log):
    out, _ = _run(q1, k1, v, q2, k2, lambda_log, trace=False)
    return out


# revision 4
# speedup vs baseline: 1.1495x; 1.1495x over previous
"""Differential attention Trainium2 kernel (Bass/Tile), 8-core SPMD.

reference:
  attn1 = softmax(causal(Q1 K1^T / sqrt(D))) V
  attn2 = softmax(causal(Q2 K2^T / sqrt(D))) V
  out   = attn1 - exp(lambda_log) * attn2
shapes: [B=2, H=12, S=2048, D=128] fp32.

Sharding: B*H = 24 head-batches, 3 per NeuronCore (data/head parallel, no
cross-core comms). Host pre-transposes Q/K to [D, S] layout so the device
needs no on-chip transposes; device returns output d-major ([D, S] per
head) and the host transposes back.

Matmul dtype strategy: all matmul operands (Q^T, K^T, V, exp-scores E,
ones) are fp16 (PE streams 1 col/cycle for 2-byte dtypes; fp32 is 4x
slower; fp8 would halve precision below the tolerance). PSUM accumulation
stays fp32 and lambda is applied exactly in fp32. Error ~4e-4 of absmax.

Device algorithm per (head, pass), in score-transposed layout:
  S_T[k, q] = matmul(lhsT=K^T_j, rhs=Q^T[q-group])      (contract D)
  E_T = exp(SCALE * S_T)  fp16   (ScalarE, PSUM->SBUF)
  out_T[d, q] += matmul(lhsT=V_j, rhs=E_T)              (contract k, PSUM acc)
  sums[128, q] += matmul(lhsT=ones128, rhs=E_T)         (denominator,
                                  pre-broadcast across all partitions)
then fin = out1_T*recip(sums1) - lam*(out2_T*recip(sums2)), muls/recip on
DVE, the final combine on the otherwise-idle GpSimd engine.

Perf structure (v2):
  - score tiles are [128,1024] fp32 = 2 PSUM banks; full key-tiles are
    processed in j-PAIRS with a single exp per pair (the ACT engine has a
    ~260ns fixed cost per instruction; halving the instruction count saves
    ~20us of ScalarE time).
  - the 4 diagonal tiles are packed into two tiles: (dr0|dr1) -> 512+384
    cols, (dr2|dr3) -> 256+128 cols, each with ONE exp and ONE two-band
    copy_predicated (bands have uniform stride inside each packed tile).
  - per-head DMA loads are split critical-first: the [0:512] slices of
    Q/K (which gate group g=0) and the first 4 V tiles land before the
    tails; the tiny const loads (ones/tri/neglam) are issued after head
    0's critical slices so they don't delay the first matmul.
"""

import sys

sys.path.insert(0, "/opt/trn_rl_repo")

import numpy as np

B, H, S, D = 2, 12, 2048, 128
NCORES = 8
BH = B * H
HEADS = BH // NCORES  # 3 heads per core
P = 128
NT = S // P           # 16 key tiles
GW = 512              # query-group width (matmul free dim)
G = S // GW           # 4 query groups
TPG = GW // P         # 4 tiles per group
SCALE = float(D) ** -0.5

_PROGRAM = None


def _build_program():
    import concourse.mybir as mybir
    import concourse.tile as tile
    from concourse import bacc

    fp32 = mybir.dt.float32
    fp16 = mybir.dt.float16
    u8 = mybir.dt.uint8
    Exp = mybir.ActivationFunctionType.Exp
    Mult = mybir.AluOpType.mult
    Add = mybir.AluOpType.add

    nc = bacc.Bacc(None)
    qt1 = nc.dram_tensor("qt1", [HEADS, P, S], fp16, kind="ExternalInput")
    kt1 = nc.dram_tensor("kt1", [HEADS, P, S], fp16, kind="ExternalInput")
    qt2 = nc.dram_tensor("qt2", [HEADS, P, S], fp16, kind="ExternalInput")
    kt2 = nc.dram_tensor("kt2", [HEADS, P, S], fp16, kind="ExternalInput")
    vd = nc.dram_tensor("v", [HEADS, P, NT, D], fp16, kind="ExternalInput")
    neglam = nc.dram_tensor("neglam", [P, 1], fp32, kind="ExternalInput")
    onesd = nc.dram_tensor("ones", [P, P], fp16, kind="ExternalInput")
    tri = nc.dram_tensor("tri", [P, P], u8, kind="ExternalInput")
    out = nc.dram_tensor("out", [HEADS, P, S], fp32, kind="ExternalOutput")

    with tile.TileContext(nc) as tc:
        with (
            tc.tile_pool(name="const", bufs=1) as cpool,
            tc.tile_pool(name="load", bufs=3) as lpool,
            tc.tile_pool(name="et", bufs=3) as epool,
            tc.tile_pool(name="fin", bufs=4) as fpool,
            tc.tile_pool(name="spsum", bufs=1, space="PSUM") as spool,
            tc.tile_pool(name="opsum", bufs=1, space="PSUM") as opool,
            tc.tile_pool(name="supsum", bufs=1, space="PSUM") as upool,
        ):
            tri_s = cpool.tile([P, P], u8)
            negbig = cpool.tile([P, P], fp32)
            nc.vector.memset(negbig[:], -1.0e30)
            neglam_s = cpool.tile([P, 1], fp32)
            ones_mat = cpool.tile([P, P], fp16)

            for h in range(HEADS):
                # critical-first loads: group g=0 needs q[0:GW], k[0:GW]
                # (diag j=0..3) and the first TPG V tiles
                qk = []
                for name, t in (("q1", qt1), ("k1", kt1), ("q2", qt2), ("k2", kt2)):
                    ts_ = lpool.tile([P, S], fp16, tag=name)
                    nc.sync.dma_start(ts_[:, 0:GW], t[h][:, 0:GW])
                    qk.append(ts_)
                v_s = lpool.tile([P, NT, D], fp16, tag="v")
                nc.sync.dma_start(v_s[:, 0:TPG, :], vd[h][:, 0:TPG, :])
                if h == 0:
                    nc.sync.dma_start(ones_mat[:], onesd[:])
                    nc.sync.dma_start(tri_s[:], tri[:])
                    nc.sync.dma_start(neglam_s[:], neglam[:])
                # tails: k first (g=1 needs k up to 1024), then v, then q
                nc.sync.dma_start(qk[1][:, GW:], kt1[h][:, GW:])
                nc.sync.dma_start(qk[3][:, GW:], kt2[h][:, GW:])
                nc.sync.dma_start(v_s[:, TPG:, :], vd[h][:, TPG:, :])
                nc.sync.dma_start(qk[0][:, GW:], qt1[h][:, GW:])
                nc.sync.dma_start(qk[2][:, GW:], qt2[h][:, GW:])

                for g in range(G):
                    jfull = TPG * g
                    qcols = [qk[2 * pi][:, g * GW : (g + 1) * GW] for pi in range(2)]
                    kss = [qk[2 * pi + 1] for pi in range(2)]
                    outp = [
                        opool.tile([P, GW], fp32, tag=f"outp{pi}", name=f"outp{pi}_{h}_{g}")
                        for pi in range(2)
                    ]
                    sums = [
                        upool.tile([P, GW], fp32, tag=f"sums{pi}", name=f"sums{pi}_{h}_{g}")
                        for pi in range(2)
                    ]

                    # Work units: ("full", (j0, j1)) -> one [128,1024] score
                    # tile (2 PSUM banks), ONE exp per pair. ("diag", (dr0,
                    # dr1)) -> the two packed diagonal tiles. Each unit is
                    # emitted per pass in two stages, software-pipelined
                    # with lag 2: stage2 (sums/PV) of unit u-2 is emitted
                    # after stage1 (QK+exp) of unit u, so the PE FIFO always
                    # holds ready matmuls while the exps of the newest units
                    # are still in flight on ScalarE.
                    units = [("full", (2 * jp, 2 * jp + 1)) for jp in range(jfull // 2)]
                    units += [("diag", (0, 1)), ("diag", (2, 3))]

                    def stage1(kind, js, pi):
                        st = spool.tile([P, 2 * GW], fp32, tag=f"st{pi}")
                        et = epool.tile([P, 2 * GW], fp16, tag=f"et{pi}")
                        if kind == "full":
                            for jj, j in enumerate(js):
                                nc.tensor.matmul(
                                    st[:, jj * GW : (jj + 1) * GW],
                                    kss[pi][:, j * P : (j + 1) * P],
                                    qcols[pi],
                                    start=True,
                                    stop=True,
                                )
                            regions = [
                                (j, None, 0, GW, jj * GW)
                                for jj, j in enumerate(js)
                            ]
                            width = 2 * GW
                        else:
                            blk = GW - js[0] * P  # width of first region
                            regions = []
                            off = 0
                            for dr in js:
                                j = jfull + dr
                                col0 = dr * P      # q offset in group
                                n = GW - col0
                                regions.append((j, dr, col0, n, off))
                                nc.tensor.matmul(
                                    st[:, off : off + n],
                                    kss[pi][:, j * P : (j + 1) * P],
                                    qk[2 * pi][:, g * GW + col0 : (g + 1) * GW],
                                    start=True,
                                    stop=True,
                                )
                                off += blk
                            # causal band: first 128 cols of each region
                            # (regions start at 0 and blk -> stride blk)
                            bands = st[:, 0 : 2 * blk].rearrange(
                                "p (b c) -> p b c", b=2, c=blk
                            )[:, :, 0:P]
                            nc.vector.copy_predicated(
                                bands,
                                tri_s[:]
                                .rearrange("p c -> p () c")
                                .broadcast_to([P, 2, P]),
                                negbig[:]
                                .rearrange("p c -> p () c")
                                .broadcast_to([P, 2, P]),
                            )
                            width = blk + (GW - js[1] * P)
                        nc.scalar.activation(
                            et[:, :width], st[:, :width], Exp, scale=SCALE
                        )
                        return et, regions

                    def stage2(kind, js, pi, et, regions):
                        for j, dr, col0, n, roff in regions:
                            ecols = et[:, roff : roff + n]
                            if kind == "full":
                                strt, stp = (j == 0), False
                            else:
                                strt = (dr == 0 and jfull == 0)
                                stp = (dr == TPG - 1)
                            nc.tensor.matmul(
                                sums[pi][:, col0:], ones_mat[:], ecols,
                                start=strt, stop=stp,
                            )
                            nc.tensor.matmul(
                                outp[pi][:, col0:], v_s[:, j, :], ecols,
                                start=strt, stop=stp,
                            )

                    pend = []
                    for kind, js in units:
                        for pi in range(2):
                            et, regions = stage1(kind, js, pi)
                            pend.append((kind, js, pi, et, regions))
                            if len(pend) > 2:
                                stage2(*pend.pop(0))
                    for u in pend:
                        stage2(*u)
                    # epilogue: rcp0 -> mul0 (frees sums0/outp0 for the next
                    # group ASAP) -> rcp1 -> mul1; combine on GpSimd
                    rcps, ts = [], []
                    for pi in range(2):
                        rcp = fpool.tile([P, GW], fp32, tag=f"rcp{pi}")
                        scr = fpool.tile([P, GW], fp32, tag="scr")
                        nc.vector.reciprocal_approx_accurate(
                            rcp[:], sums[pi][:], scr[:]
                        )
                        t_ = fpool.tile([P, GW], fp32, tag=f"t{pi}")
                        nc.vector.tensor_mul(t_[:], outp[pi][:], rcp[:])
                        rcps.append(rcp)
                        ts.append(t_)
                    fin = fpool.tile([P, GW], fp32, tag="fin")
                    # fin = t0 - lam*t1  (lam exact in fp32 via neglam column)
                    nc.vector.scalar_tensor_tensor(
                        fin[:], ts[1][:], neglam_s[:], ts[0][:],
                        op0=Mult, op1=Add,
                    )
                    nc.sync.dma_start(out[h][:, g * GW : (g + 1) * GW], fin[:])

    nc.compile()
    return nc


def _get_program():
    global _PROGRAM
    if _PROGRAM is None:
        _PROGRAM = _build_program()
    return _PROGRAM


def _make_in_maps(q1, k1, v, q2, k2, lambda_log):
    lam_val = float(np.exp(np.float64(lambda_log.reshape(-1)[0])))
    neglam_np = np.full((P, 1), -lam_val, dtype=np.float32)
    ones_np = np.ones((P, P), dtype=np.float16)
    # kill-mask for the diagonal band: 1 where k > q (strictly below diag)
    tri_np = (np.arange(P)[:, None] > np.arange(P)[None, :]).astype(np.uint8)

    def t(x, dt_):  # [BH, S, D] -> [BH, D, S] contiguous
        return np.ascontiguousarray(
            x.reshape(BH, S, D).transpose(0, 2, 1)
        ).astype(dt_)

    q1t = t(q1, np.float16)
    q2t = t(q2, np.float16)
    k1t = t(k1, np.float16)
    k2t = t(k2, np.float16)
    # pre-tile V to [BH, p, j, d] so the SBUF load is contiguous per
    # partition: v_s[p, j, d] = V[128 j + p, d]
    vf = np.ascontiguousarray(
        v.reshape(BH, NT, P, D).transpose(0, 2, 1, 3)
    ).astype(np.float16)

    in_maps = []
    for c in range(NCORES):
        sl = slice(c * HEADS, (c + 1) * HEADS)
        in_maps.append(
            {
                "qt1": q1t[sl],
                "kt1": k1t[sl],
                "qt2": q2t[sl],
                "kt2": k2t[sl],
                "v": vf[sl],
                "neglam": neglam_np,
                "ones": ones_np,
                "tri": tri_np,
            }
        )
    return in_maps


def _run(q1, k1, v, q2, k2, lambda_log, trace=False):
    from concourse.bass_utils import run_bass_kernel_spmd

    nc = _get_program()
    in_maps = _make_in_maps(q1, k1, v, q2, k2, lambda_log)
    res = run_bass_kernel_spmd(
        nc, in_maps, core_ids=list(range(NCORES)), trace=trace
    )
    parts = [res.results[c]["out"].transpose(0, 2, 1) for c in range(NCORES)]
    full = np.concatenate(parts, axis=0).reshape(B, H, S, D)
    return np.ascontiguousarray(full, dtype=np.float32), res


def kernel(q1, k1, v, q2, k2, lambda_log):
    out, _ = _run(q1, k1, v, q2, k2, lambda_log, trace=False)
    return out


# revision 6
# speedup vs baseline: 1.2468x; 1.0846x over previous
"""Differential attention Trainium2 kernel (Bass/Tile), 8-core SPMD.

reference:
  attn1 = softmax(causal(Q1 K1^T / sqrt(D))) V
  attn2 = softmax(causal(Q2 K2^T / sqrt(D))) V
  out   = attn1 - exp(lambda_log) * attn2
shapes: [B=2, H=12, S=2048, D=128] fp32.

Sharding: B*H = 24 head-batches, 3 per NeuronCore (data/head parallel, no
cross-core comms). Host pre-transposes Q/K to [D, S] layout so the device
needs no on-chip transposes; device returns output d-major ([D, S] per
head) and the host transposes back.

Matmul dtype strategy: all matmul operands (Q^T, K^T, V, exp-scores E,
ones) are fp16 (PE streams 1 col/cycle for 2-byte dtypes; fp32 is 4x
slower). PSUM accumulation stays fp32, lambda applied exactly in fp32.
Error ~4e-4 of output absmax.

Device algorithm per (head, pass), in score-transposed layout:
  S_T[k, q] = matmul(lhsT=K^T_j, rhs=Q^T[q-group])      (contract D)
  E_T = exp(SCALE * S_T)  fp16   (ScalarE, PSUM->SBUF)
  out_T[d, q] += matmul(lhsT=V_j, rhs=E_T)              (contract k, PSUM acc)
  sums[128, q] += matmul(lhsT=ones128, rhs=E_T)         (denominator,
                                  pre-broadcast across all partitions)
then fin = out1_T*recip(sums1) - lam*(out2_T*recip(sums2)) on DVE.

Perf structure (v3):
  - score tiles are [128,1024] fp32 = 2 PSUM banks; full key-tiles are
    processed in j-PAIRS with a single exp per pair (ScalarE has ~260ns
    fixed cost per instruction). The 4 diagonal tiles are packed into two
    tiles, (dr0|dr1) and (dr2|dr3), each with ONE exp and ONE two-band
    copy_predicated (uniform band stride inside each packed tile).
  - ONE flat software pipeline across all (head, group) work: each unit
    (score-tile, pass) is emitted in two stages -- stage1 = QK matmuls
    [+ band mask] + exp, stage2 = sums/PV matmuls -- with a global lag of
    4 units between them.  This keeps ready matmuls in the PE's strict
    FIFO while exps are in flight on ScalarE, and (crucially) emits the
    next group's copy_predicated ops on the strict-FIFO Vector engine
    BEFORE the previous group's ~5us reciprocal/normalize epilogue, so
    the exp chain is never blocked behind the epilogue at group/head
    boundaries (this was a measured 4us PE stall per head + PE clock
    re-throttle).
  - per-pass epilogue halves (reciprocal + multiply) are emitted as soon
    as that pass's accumulation stops, shortening the serial tail.
  - per-head DMA loads are critical-first (k[0:128]+q[0:512] land first,
    so the first QK can start ~3us into the kernel) and the next head's
    loads are hoisted to the start of the previous head's last group.
"""

import sys

sys.path.insert(0, "/opt/trn_rl_repo")

import numpy as np

B, H, S, D = 2, 12, 2048, 128
NCORES = 8
BH = B * H
HEADS = BH // NCORES  # 3 heads per core
P = 128
NT = S // P           # 16 key tiles
GW = 512              # query-group width (matmul free dim)
G = S // GW           # 4 query groups
TPG = GW // P         # 4 tiles per group
SCALE = float(D) ** -0.5
LAG = 4               # software-pipeline depth, in (tile, pass) units

_PROGRAM = None


def _build_program():
    import concourse.mybir as mybir
    import concourse.tile as tile
    from concourse import bacc

    fp32 = mybir.dt.float32
    fp16 = mybir.dt.float16
    u8 = mybir.dt.uint8
    Exp = mybir.ActivationFunctionType.Exp
    Mult = mybir.AluOpType.mult
    Add = mybir.AluOpType.add

    nc = bacc.Bacc(None)
    qt1 = nc.dram_tensor("qt1", [HEADS, P, S], fp16, kind="ExternalInput")
    kt1 = nc.dram_tensor("kt1", [HEADS, P, S], fp16, kind="ExternalInput")
    qt2 = nc.dram_tensor("qt2", [HEADS, P, S], fp16, kind="ExternalInput")
    kt2 = nc.dram_tensor("kt2", [HEADS, P, S], fp16, kind="ExternalInput")
    vd = nc.dram_tensor("v", [HEADS, P, NT, D], fp16, kind="ExternalInput")
    neglam = nc.dram_tensor("neglam", [P, 1], fp32, kind="ExternalInput")
    onesd = nc.dram_tensor("ones", [P, P], fp16, kind="ExternalInput")
    tri = nc.dram_tensor("tri", [P, P], u8, kind="ExternalInput")
    out = nc.dram_tensor("out", [HEADS, P, S], fp32, kind="ExternalOutput")

    with tile.TileContext(nc) as tc:
        with (
            tc.tile_pool(name="const", bufs=1) as cpool,
            tc.tile_pool(name="load", bufs=3) as lpool,
            tc.tile_pool(name="et", bufs=3) as epool,
            tc.tile_pool(name="fin", bufs=4) as fpool,
            tc.tile_pool(name="spsum", bufs=1, space="PSUM") as spool,
            tc.tile_pool(name="opsum", bufs=1, space="PSUM") as opool,
            tc.tile_pool(name="supsum", bufs=1, space="PSUM") as upool,
        ):
            tri_s = cpool.tile([P, P], u8)
            negbig = cpool.tile([P, P], fp32)
            nc.vector.memset(negbig[:], -1.0e30)
            neglam_s = cpool.tile([P, 1], fp32)
            ones_mat = cpool.tile([P, P], fp16)

            head_tiles = {}

            def emit_loads(h):
                # critical-first: the first diag QKs need k[0:128] and
                # q[0:512]; tri gates the first copy_predicated
                qk = [
                    lpool.tile([P, S], fp16, tag=n, name=f"{n}_{h}")
                    for n in ("q1", "k1", "q2", "k2")
                ]
                v_s = lpool.tile([P, NT, D], fp16, tag="v", name=f"v_{h}")
                nc.sync.dma_start(qk[1][:, 0:P], kt1[h][:, 0:P])
                nc.sync.dma_start(qk[0][:, 0:GW], qt1[h][:, 0:GW])
                if h == 0:
                    nc.sync.dma_start(tri_s[:], tri[:])
                nc.sync.dma_start(qk[3][:, 0:P], kt2[h][:, 0:P])
                nc.sync.dma_start(qk[2][:, 0:GW], qt2[h][:, 0:GW])
                nc.sync.dma_start(qk[1][:, P:GW], kt1[h][:, P:GW])
                nc.sync.dma_start(qk[3][:, P:GW], kt2[h][:, P:GW])
                if h == 0:
                    nc.sync.dma_start(ones_mat[:], onesd[:])
                    nc.sync.dma_start(neglam_s[:], neglam[:])
                nc.sync.dma_start(v_s[:, 0:TPG, :], vd[h][:, 0:TPG, :])
                nc.sync.dma_start(qk[1][:, GW:], kt1[h][:, GW:])
                nc.sync.dma_start(qk[3][:, GW:], kt2[h][:, GW:])
                nc.sync.dma_start(v_s[:, TPG:, :], vd[h][:, TPG:, :])
                nc.sync.dma_start(qk[0][:, GW:], qt1[h][:, GW:])
                nc.sync.dma_start(qk[2][:, GW:], qt2[h][:, GW:])
                head_tiles[h] = (qk, v_s)

            # flat unit list across heads/groups; each unit = (score tile,
            # pass).  Group units: jfull//2 full pairs + 2 packed diag tiles.
            units = []
            for h in range(HEADS):
                for g in range(G):
                    gu = [("full", (2 * jp, 2 * jp + 1)) for jp in range(TPG * g // 2)]
                    gu += [("diag", (0, 1)), ("diag", (2, 3))]
                    for kind, js in gu:
                        for pi in range(2):
                            units.append((h, g, kind, js, pi))
            n_units = len(units)
            first_of_group = {}
            last_of_pass = {}
            for idx, (h, g, kind, js, pi) in enumerate(units):
                first_of_group.setdefault((h, g), idx)
                last_of_pass[(h, g, pi)] = idx

            ctxs = {}

            def stage1(idx):
                h, g, kind, js, pi = units[idx]
                if idx == first_of_group.get((h, 0), -1):
                    emit_loads(h)
                if h + 1 < HEADS and idx == first_of_group[(h, G - 1)]:
                    emit_loads(h + 1)  # prefetch next head early
                if (h, g) not in ctxs:
                    ctxs[(h, g)] = {
                        "outp": [
                            opool.tile([P, GW], fp32, tag=f"outp{p_}",
                                       name=f"outp{p_}_{h}_{g}")
                            for p_ in range(2)
                        ],
                        "sums": [
                            upool.tile([P, GW], fp32, tag=f"sums{p_}",
                                       name=f"sums{p_}_{h}_{g}")
                            for p_ in range(2)
                        ],
                        "t": [None, None],
                    }
                qk, v_s = head_tiles[h]
                jfull = TPG * g
                st = spool.tile([P, 2 * GW], fp32, tag=f"st{pi}")
                et = epool.tile([P, 2 * GW], fp16, tag=f"et{pi}")
                ks = qk[2 * pi + 1]
                if kind == "full":
                    for jj, j in enumerate(js):
                        nc.tensor.matmul(
                            st[:, jj * GW : (jj + 1) * GW],
                            ks[:, j * P : (j + 1) * P],
                            qk[2 * pi][:, g * GW : (g + 1) * GW],
                            start=True,
                            stop=True,
                        )
                    regions = [(j, None, 0, GW, jj * GW) for jj, j in enumerate(js)]
                    width = 2 * GW
                else:
                    blk = GW - js[0] * P  # width of first region
                    regions = []
                    off = 0
                    for dr in js:
                        j = jfull + dr
                        col0 = dr * P      # q offset in group
                        n = GW - col0
                        regions.append((j, dr, col0, n, off))
                        nc.tensor.matmul(
                            st[:, off : off + n],
                            ks[:, j * P : (j + 1) * P],
                            qk[2 * pi][:, g * GW + col0 : (g + 1) * GW],
                            start=True,
                            stop=True,
                        )
                        off += blk
                    # causal band: first 128 cols of each region (regions
                    # start at 0 and blk -> uniform stride)
                    bands = st[:, 0 : 2 * blk].rearrange(
                        "p (b c) -> p b c", b=2, c=blk
                    )[:, :, 0:P]
                    nc.vector.copy_predicated(
                        bands,
                        tri_s[:].rearrange("p c -> p () c").broadcast_to([P, 2, P]),
                        negbig[:].rearrange("p c -> p () c").broadcast_to([P, 2, P]),
                    )
                    width = blk + (GW - js[1] * P)
                nc.scalar.activation(et[:, :width], st[:, :width], Exp, scale=SCALE)
                return (idx, et, regions)

            def stage2(idx, et, regions):
                h, g, kind, js, pi = units[idx]
                ctx = ctxs[(h, g)]
                jfull = TPG * g
                _, v_s = head_tiles[h]
                for j, dr, col0, n, roff in regions:
                    ecols = et[:, roff : roff + n]
                    if kind == "full":
                        strt, stp = (j == 0), False
                    else:
                        strt = (dr == 0 and jfull == 0)
                        stp = (dr == TPG - 1)
                    nc.tensor.matmul(
                        ctx["sums"][pi][:, col0:], ones_mat[:], ecols,
                        start=strt, stop=stp,
                    )
                    nc.tensor.matmul(
                        ctx["outp"][pi][:, col0:], v_s[:, j, :], ecols,
                        start=strt, stop=stp,
                    )
                # per-pass epilogue half as soon as this pass's accumulation
                # is complete; combine + store once both halves are done
                if idx == last_of_pass[(h, g, pi)]:
                    rcp = fpool.tile([P, GW], fp32, tag=f"rcp{pi}")
                    scr = fpool.tile([P, GW], fp32, tag=f"scr{pi}")
                    nc.vector.reciprocal_approx_accurate(
                        rcp[:], ctx["sums"][pi][:], scr[:]
                    )
                    t_ = fpool.tile([P, GW], fp32, tag=f"t{pi}")
                    nc.vector.tensor_mul(t_[:], ctx["outp"][pi][:], rcp[:])
                    ctx["t"][pi] = t_
                    if all(x is not None for x in ctx["t"]):
                        fin = fpool.tile([P, GW], fp32, tag="fin")
                        # fin = t0 - lam*t1 (lam exact in fp32)
                        nc.vector.scalar_tensor_tensor(
                            fin[:], ctx["t"][1][:], neglam_s[:], ctx["t"][0][:],
                            op0=Mult, op1=Add,
                        )
                        nc.sync.dma_start(
                            out[h][:, g * GW : (g + 1) * GW], fin[:]
                        )
                        del ctxs[(h, g)]

            pend = []
            for idx in range(n_units):
                pend.append(stage1(idx))
                if len(pend) > LAG:
                    stage2(*pend.pop(0))
            for u in pend:
                stage2(*u)

    nc.compile()
    return nc


def _get_program():
    global _PROGRAM
    if _PROGRAM is None:
        _PROGRAM = _build_program()
    return _PROGRAM


def _make_in_maps(q1, k1, v, q2, k2, lambda_log):
    lam_val = float(np.exp(np.float64(lambda_log.reshape(-1)[0])))
    neglam_np = np.full((P, 1), -lam_val, dtype=np.float32)
    ones_np = np.ones((P, P), dtype=np.float16)
    # kill-mask for the diagonal band: 1 where k > q (strictly below diag)
    tri_np = (np.arange(P)[:, None] > np.arange(P)[None, :]).astype(np.uint8)

    def t(x, dt_):  # [BH, S, D] -> [BH, D, S] contiguous
        return np.ascontiguousarray(
            x.reshape(BH, S, D).transpose(0, 2, 1)
        ).astype(dt_)

    q1t = t(q1, np.float16)
    q2t = t(q2, np.float16)
    k1t = t(k1, np.float16)
    k2t = t(k2, np.float16)
    # pre-tile V to [BH, p, j, d] so the SBUF load is contiguous per
    # partition: v_s[p, j, d] = V[128 j + p, d]
    vf = np.ascontiguousarray(
        v.reshape(BH, NT, P, D).transpose(0, 2, 1, 3)
    ).astype(np.float16)

    in_maps = []
    for c in range(NCORES):
        sl = slice(c * HEADS, (c + 1) * HEADS)
        in_maps.append(
            {
                "qt1": q1t[sl],
                "kt1": k1t[sl],
                "qt2": q2t[sl],
                "kt2": k2t[sl],
                "v": vf[sl],
                "neglam": neglam_np,
                "ones": ones_np,
                "tri": tri_np,
            }
        )
    return in_maps


def _run(q1, k1, v, q2, k2, lambda_log, trace=False):
    from concourse.bass_utils import run_bass_kernel_spmd

    nc = _get_program()
    in_maps = _make_in_maps(q1, k1, v, q2, k2, lambda_log)
    res = run_bass_kernel_spmd(
        nc, in_maps, core_ids=list(range(NCORES)), trace=trace
    )
    parts = [res.results[c]["out"].transpose(0, 2, 1) for c in range(NCORES)]
    full = np.concatenate(parts, axis=0).reshape(B, H, S, D)
    return np.ascontiguousarray(full, dtype=np.float32), res


def kernel(q1, k1, v, q2, k2, lambda_log):
    out, _ = _run(q1, k1, v, q2, k2, lambda_log, trace=False)
    return out
